# revision 9
# baseline (speedup 1.0000x reference)
"""HSTU-style 4-layer transformer (B=8, T=2048, D=128, H=2) on 8 Trainium2 cores.

Data-parallel over batch: each NeuronCore runs one full sequence.
Residual stream kept feature-major [D=128 partitions, T=2048 free].

Fully chunk-fused pipeline: for each 512-token chunk j the kernel emits
ln1 -> U/Q/K/V proj -> attention -> hstu-norm -> f2 -> ln2 -> FFN in one
stream, so attention matmuls of chunk j+1 overlap the norm/FFN tail of
chunk j (and layer boundaries pipeline chunk-wise too).

Weights/activations bf16 (1 cycle/row matmuls at any tile size), causal
mask fused into the A2 clamp on DVE (no PE mask matmuls), V written by
the Silu activation directly into the interleaved [64|1|64|1] layout
(denominator rides as a 65th lhsT row), gelu evaluated off the silu
table as silu(1.702x)/1.702 (c2w pre-scaled on host) so the scalar
engine loads exactly one activation table for the whole kernel.
"""
import numpy as np
from contextlib import ExitStack

import concourse.bass as bass
import concourse.tile as tile
from concourse import bacc, mybir
from concourse._compat import with_exitstack
from concourse.alu_op_type import AluOpType
from concourse.masks import make_identity

F32 = mybir.dt.float32
F32R = mybir.dt.float32r
BF16 = mybir.dt.bfloat16
I32 = mybir.dt.int32
AF = mybir.ActivationFunctionType
MULT = AluOpType.mult
ADD = AluOpType.add
MAX = AluOpType.max

B, T, D, L, H = 8, 2048, 128, 4, 2
HD = D // H
NITEMS = 200000
EPS = 1e-8
SCALE = 1.0 / np.sqrt(HD)
GELU_A = 1.702
NT = T // 512          # 4 t-chunks of 512
NS = T // 128          # 16 s-chunks of 128
QUAKE_C = 0x5F3759DF


def _quake_rsqrt(nc, pool, v, out_dtype, tag, prow=None):
    """1/sqrt(v) elementwise on DVE: quake seed + 2 Newton iterations.
    v: AP over partitions prow (or all), fp32 SBUF, strictly positive.
    Internal tiles are [128, n]; ops run on the prow slice so all operands
    share a base partition. Returns the final [128, n] tile (valid at prow)."""
    n = v.shape[-1]
    if prow is None:
        prow = slice(0, 128)
    q1 = pool.tile([128, n], I32, tag=f"{tag}_q1")
    nc.vector.tensor_scalar(out=q1[prow, :], in0=v.bitcast(I32), scalar1=1.0,
                            scalar2=None, op0=AluOpType.logical_shift_right)
    q2 = pool.tile([128, n], I32, tag=f"{tag}_q2")
    nc.vector.tensor_scalar(out=q2[prow, :], in0=q1[prow, :], scalar1=-1.0,
                            scalar2=float(QUAKE_C), op0=MULT, op1=ADD)
    cur = q2.bitcast(F32)
    for it in range(2):
        sq = pool.tile([128, n], F32, tag=f"{tag}_sq{it}")
        nc.vector.tensor_tensor(sq[prow, :], cur[prow, :], cur[prow, :], op=MULT)
        hv = pool.tile([128, n], F32, tag=f"{tag}_hv{it}")
        nc.vector.scalar_tensor_tensor(out=hv[prow, :], in0=v, scalar=-0.5,
                                       in1=sq[prow, :], op0=MULT, op1=MULT)
        w_ = pool.tile([128, n], F32, tag=f"{tag}_w{it}")
        nc.vector.tensor_scalar(out=w_[prow, :], in0=hv[prow, :], scalar1=1.5,
                                scalar2=None, op0=ADD)
        nxt = pool.tile([128, n], out_dtype if it == 1 else F32, tag=f"{tag}_y{it}")
        nc.vector.tensor_tensor(nxt[prow, :], cur[prow, :], w_[prow, :], op=MULT)
        cur = nxt
    return cur


@with_exitstack
def _build(ctx: ExitStack, tc: tile.TileContext, io, vb_nonzero: bool):
    nc = tc.nc
    cst = ctx.enter_context(tc.tile_pool(name="cst", bufs=1))
    big = ctx.enter_context(tc.tile_pool(name="big", bufs=2))
    sA = ctx.enter_context(tc.tile_pool(name="sA", bufs=3))
    gat = ctx.enter_context(tc.tile_pool(name="gat", bufs=3))
    st = ctx.enter_context(tc.tile_pool(name="st", bufs=2))
    ps_S = ctx.enter_context(tc.tile_pool(name="ps_S", bufs=2, space="PSUM"))
    ps_av = ctx.enter_context(tc.tile_pool(name="ps_av", bufs=1, space="PSUM"))
    ps_b = ctx.enter_context(tc.tile_pool(name="ps_b", bufs=2, space="PSUM"))

    # ---- load constants / weights ----
    ident = cst.tile([128, 128], F32)
    make_identity(nc, ident)

    wB = {}
    for nm in ("wq", "wk", "wu", "wv", "wf2", "wc1", "wc2"):
        t = cst.tile([128, L * 128], BF16, tag=f"{nm}_t")
        nc.sync.dma_start(t.rearrange("p (l m) -> p l m", l=L),
                          io[nm].rearrange("l k m -> k l m"))
        wB[nm] = t

    sel2_f = cst.tile([2, 128], F32)
    nc.sync.dma_start(sel2_f, io["sel2"])
    sel2 = cst.tile([2, 128], F32R)
    nc.vector.tensor_copy(sel2, sel2_f)
    ones1_f = cst.tile([1, 128], F32)
    nc.sync.dma_start(ones1_f, io["ones1"])
    ones1 = cst.tile([1, 128], F32R)
    nc.vector.tensor_copy(ones1, ones1_f)
    onesc_f = cst.tile([128, 1], F32)
    nc.sync.dma_start(onesc_f, io["onesc"])
    onesc = cst.tile([128, 1], F32R)
    nc.vector.tensor_copy(onesc, onesc_f)
    ones2t_f = cst.tile([128, 2], F32)
    nc.sync.dma_start(ones2t_f, io["ones2t"])
    ones2t = cst.tile([128, 2], F32R)
    nc.vector.tensor_copy(ones2t, ones2t_f)

    # causal keep-mask for diag blocks: M[p, c] = 1 if c >= p else 0
    Mtri = cst.tile([128, 512], BF16)
    nc.sync.dma_start(Mtri, io["mtri"])

    posT = cst.tile([128, T], F32)
    nc.sync.dma_start(posT, io["posT"])
    idx = cst.tile([128, NS], I32)
    nc.sync.dma_start(idx, io["idx"])
    emb_s = cst.tile([128, 1], F32)
    nc.sync.dma_start(emb_s, io["emb_s"])
    last_s = cst.tile([128, 1], F32)
    nc.sync.dma_start(last_s, io["last_s"])
    bcol = {}
    for nm in ("ub", "qb", "kb", "c1b", "f2b", "c2b"):
        bt = cst.tile([128, L], F32, tag=f"{nm}_t")
        nc.sync.dma_start(bt, io[nm].rearrange("l k -> k l"))
        bcol[nm] = bt
    if vb_nonzero:
        vbB = cst.tile([128, L * 128], F32, tag="vbB")
        nc.sync.dma_start(vbB.rearrange("p (l m) -> p l m", l=L),
                          io["vbB"].rearrange("l p m -> p l m"))

    # ---- helper: per-chunk rms rstd ----
    # x_sb[:, jc] f32 -> writes rstd into rrow (a [1, T]-slice AP) at cols jc
    def ln_chunk(x_sb, j, pdt, rrow):
        jc = slice(j * 512, (j + 1) * 512)
        p32 = slice(32 * j, 32 * (j + 1))
        xsq = st.tile([128, 512], F32R, tag="ln_xsq")
        nc.vector.tensor_tensor(xsq, x_sb[:, jc], x_sb[:, jc], op=MULT)
        mp = ps_b.tile([1, 512], F32, tag="pb")
        nc.tensor.matmul(mp, onesc, xsq, start=True, stop=True)
        row = st.tile([1, 512], F32, tag="ln_row")
        nc.vector.tensor_copy(row, mp)
        nc.sync.dma_start(pdt[p32, :], row)
        mi = st.tile([128, 16], F32, tag="ln_mi")
        nc.vector.tensor_scalar(out=mi[p32, :], in0=pdt[p32, :], scalar1=1.0 / D,
                                scalar2=EPS, op0=MULT, op1=ADD)
        rs = _quake_rsqrt(nc, st, mi[p32, :], F32R, "lnq", prow=p32)
        nc.sync.dma_start(rrow[:, jc], rs[p32, :])

    def bcast_row(rrow, j):
        """K=1 broadcast matmul: row [1, T] F32R slice cols j*512.. -> psum."""
        bp = ps_b.tile([128, 512], F32, tag="pb")
        nc.tensor.matmul(bp, ones1, rrow[:, j * 512:(j + 1) * 512],
                         start=True, stop=True)
        return bp

    # ================= embedding gather + transpose + pos (chunk-wise) ====
    e_sb = big.tile([128, T], F32, tag="e", bufs=1)
    x_sb = big.tile([128, T], F32, tag="xA")
    erow = st.tile([1, T], F32R, tag="r1row")
    pde = st.tile([128, 16], F32, tag="pde")
    for g in range(4):
        tr_ps = ps_b.tile([128, 512], F32, tag="pb")
        for c4 in range(4):
            c = 4 * g + c4
            tok = gat.tile([128, 128], F32, tag="tok")
            nc.gpsimd.indirect_dma_start(
                out=tok, out_offset=None, in_=io["itab"][:, :],
                in_offset=bass.IndirectOffsetOnAxis(ap=idx[:, c:c + 1], axis=0))
            nc.tensor.transpose(tr_ps[:, c4 * 128:(c4 + 1) * 128], tok, ident)
        gc = slice(g * 512, (g + 1) * 512)
        nc.vector.tensor_tensor(e_sb[:, gc], tr_ps, posT[:, gc], op=ADD)
        ln_chunk(e_sb, g, pde, erow)
        bp = bcast_row(erow, g)
        nc.vector.scalar_tensor_tensor(
            out=x_sb[:, gc], in0=bp, scalar=emb_s[:, 0:1],
            in1=e_sb[:, gc], op0=MULT, op1=MULT)

    # ================= layers (fully chunk-fused) =================
    for l in range(L):
        lw = slice(l * 128, (l + 1) * 128)

        xn = big.tile([128, T], BF16, tag="xn")
        U = big.tile([128, T], BF16, tag="U")
        Q = big.tile([128, T], BF16, tag="Q")
        K = big.tile([128, T], BF16, tag="K")
        v130 = big.tile([128, NS * 130], BF16, tag="v130")
        AVU = big.tile([128, T], BF16, tag="AVU", bufs=1)
        x2 = big.tile([128, T], F32, tag="x2")
        xn2 = big.tile([128, T], BF16, tag="xn2")
        hh = big.tile([128, T], BF16, tag="hh", bufs=1)
        x3 = big.tile([128, T], F32, tag="xB" if l % 2 == 0 else "xA")
        r1row = st.tile([1, T], F32R, tag="r1row")
        r2row = st.tile([1, T], F32R, tag="r2row", bufs=1)
        GGrow = st.tile([2, T], F32R, tag="GGrow", bufs=1)
        GG0 = GGrow[0:1, :]
        GG1 = GGrow[1:2, :]
        pd1 = st.tile([128, 16], F32, tag="pd1")
        pd2 = st.tile([128, 16], F32, tag="pd2")
        pdh = st.tile([128, 64], F32, tag="pdh")

        # ones columns of v130: [*, i*130 + {64, 129}] = 1
        ones_ap = bass.AP(tensor=v130.tensor, offset=v130.offset + 64,
                          ap=[v130.ap[0], [130, NS], [65, 2], [1, 1]])
        nc.gpsimd.memset(ones_ap, 1.0)

        for j in range(NT):
            jc = slice(j * 512, (j + 1) * 512)
            p32 = slice(32 * j, 32 * (j + 1))

            # ---- ln1(j) + xn(j) ----
            ln_chunk(x_sb, j, pd1, r1row)
            bp = bcast_row(r1row, j)
            nc.vector.tensor_tensor(xn[:, jc], bp, x_sb[:, jc], op=MULT)

            # ---- U/Q/K projections (feature-major) ----
            for nm, dst in (("wu", U), ("wq", Q), ("wk", K)):
                bnm = {"wu": "ub", "wq": "qb", "wk": "kb"}[nm]
                up = ps_b.tile([128, 512], F32, tag="pb")
                nc.tensor.matmul(up, wB[nm][:, lw], xn[:, jc], start=True, stop=True)
                nc.scalar.activation(dst[:, jc], up, AF.Silu,
                                     bias=bcol[bnm][:, l:l + 1], scale=1.0)

            # ---- V projection (token-major) + silu direct into v130 ----
            vp = ps_b.tile([128, 512], F32, tag="pb")
            for c4 in range(4):
                c = 4 * j + c4
                nc.tensor.matmul(vp[:, c4 * 128:(c4 + 1) * 128],
                                 xn[:, c * 128:(c + 1) * 128], wB["wv"][:, lw],
                                 start=True, stop=True)
            if vb_nonzero:
                vb_ap = bass.AP(tensor=vbB.tensor, offset=vbB.offset + l * 128,
                                ap=[vbB.ap[0], [0, 4], [1, 128]])
                vtmp = st.tile([128, 512], F32, tag="vtmp")
                nc.vector.tensor_tensor(vtmp, vp, vb_ap, op=ADD)
                vsrc = vtmp
            else:
                vsrc = vp
            # dst AP: (c4, h, 64) -> col 130*(4j+c4) + 65*h + d
            vdst = bass.AP(tensor=v130.tensor, offset=v130.offset + j * 4 * 130,
                           ap=[v130.ap[0], [130, 4], [65, 2], [1, 64]])
            vsrc_ap = bass.AP(tensor=vsrc.tensor, offset=vsrc.offset,
                              ap=[vsrc.ap[0], [128, 4], [64, 2], [1, 64]])
            nc.scalar.activation(vdst, vsrc_ap, AF.Silu)

            # ---- attention chunk j ----
            avb = ps_av.tile([128, 1024], F32, tag="avb")
            nsc = 4 * (j + 1)
            for i in range(nsc):
                Sp = ps_S.tile([128, 1024], F32, tag="S")
                diag = i >= 4 * j
                off = 128 * (i - 4 * j) if diag else 0
                tq = slice(j * 512 + off, (j + 1) * 512)
                s0 = slice(off, 512)
                s1 = slice(512 + off, 1024)
                nc.tensor.matmul(Sp[:, s0], K[0:64, i * 128:(i + 1) * 128],
                                 Q[0:64, tq], start=True, stop=True)
                nc.tensor.matmul(Sp[:, s1], K[64:128, i * 128:(i + 1) * 128],
                                 Q[64:128, tq], start=True, stop=True)
                A = sA.tile([128, 1024], BF16, tag="A")
                A2 = sA.tile([128, 1024], BF16, tag="A2")
                if diag:
                    w = 512 - off
                    nc.scalar.activation(A[:, s0], Sp[:, s0], AF.Silu, scale=SCALE)
                    nc.scalar.activation(A[:, s1], Sp[:, s1], AF.Silu, scale=SCALE)
                    nc.vector.scalar_tensor_tensor(
                        out=A2[:, s0], in0=A[:, s0], scalar=0.0,
                        in1=Mtri[:, 0:w], op0=MAX, op1=MULT)
                    nc.vector.scalar_tensor_tensor(
                        out=A2[:, s1], in0=A[:, s1], scalar=0.0,
                        in1=Mtri[:, 0:w], op0=MAX, op1=MULT)
                else:
                    nc.scalar.activation(A, Sp, AF.Silu, scale=SCALE)
                    nc.vector.tensor_scalar_max(A2, A, 0.0)
                nc.tensor.matmul(avb[0:65, s0], v130[:, i * 130:i * 130 + 65],
                                 A2[:, s0], start=(i == 0), stop=(i == nsc - 1))
                nc.tensor.matmul(avb[0:65, s1], v130[:, i * 130 + 65:i * 130 + 130],
                                 A2[:, s1], start=(i == 0), stop=(i == nsc - 1))

            # ---- drain AV + hstu-norm stats ----
            nc.vector.tensor_tensor(AVU[0:64, jc], avb[0:64, 0:512], U[0:64, jc], op=MULT)
            nc.vector.tensor_tensor(AVU[64:128, jc], avb[0:64, 512:1024],
                                    U[64:128, jc], op=MULT)
            avsq = st.tile([128, 512], F32R, tag="avsq")
            nc.scalar.activation(avsq[0:64, :], avb[0:64, 0:512], AF.Square)
            nc.scalar.activation(avsq[64:128, :], avb[0:64, 512:1024], AF.Square)
            ssq_ps = ps_b.tile([2, 512], F32, tag="pb")
            nc.tensor.matmul(ssq_ps, ones2t, avsq, start=True, stop=True)
            drow = st.tile([1, 1024], F32, tag="drow", bufs=1)
            nc.vector.tensor_copy(drow, avb[64:65, :])
            sqr = st.tile([2, 512], F32, tag="sqr")
            nc.vector.tensor_copy(sqr, ssq_ps)
            nc.sync.dma_start(pdh[p32, 0:16], drow[:, 0:512])
            nc.sync.dma_start(pdh[p32, 16:32], drow[:, 512:1024])
            nc.sync.dma_start(pdh[p32, 32:48], sqr[0:1, :])
            nc.sync.dma_start(pdh[p32, 48:64], sqr[1:2, :])

            # hstu norm scales for this chunk (all tiles sliced at p32)
            de = st.tile([128, 32], F32, tag="hde")
            nc.vector.tensor_scalar(out=de[p32, :], in0=pdh[p32, 0:32], scalar1=EPS,
                                    scalar2=None, op0=ADD)
            rr = st.tile([128, 32], F32, tag="hrr")
            scr = st.tile([128, 32], F32, tag="hscr")
            nc.vector.reciprocal_approx_accurate(rr, de, scratch=scr)
            r2 = st.tile([128, 32], F32, tag="hr2")
            nc.vector.tensor_tensor(r2[p32, :], rr[p32, :], rr[p32, :], op=MULT)
            uu = st.tile([128, 32], F32, tag="huu")
            nc.vector.tensor_tensor(uu[p32, :], r2[p32, :], pdh[p32, 32:64], op=MULT)
            mm_ = st.tile([128, 16], F32, tag="hmm")
            nc.vector.tensor_tensor(mm_[p32, :], uu[p32, 0:16], uu[p32, 16:32], op=ADD)
            mi = st.tile([128, 16], F32, tag="hmi")
            nc.vector.tensor_scalar(out=mi[p32, :], in0=mm_[p32, :], scalar1=1.0 / D,
                                    scalar2=EPS, op0=MULT, op1=ADD)
            Rq = _quake_rsqrt(nc, st, mi[p32, :], F32, "hq", prow=p32)
            GG = st.tile([128, 32], F32R, tag="GG")
            nc.vector.tensor_tensor(GG[p32, 0:16], rr[p32, 0:16], Rq[p32, :], op=MULT)
            nc.vector.tensor_tensor(GG[p32, 16:32], rr[p32, 16:32], Rq[p32, :], op=MULT)
            nc.sync.dma_start(GGrow[0:1, jc], GG[p32, 0:16])
            nc.sync.dma_start(GGrow[1:2, jc], GG[p32, 16:32])

            # ---- f2 + residual ----
            gb = ps_b.tile([128, 512], F32, tag="pb")
            nc.tensor.matmul(gb, sel2, GGrow[:, jc], start=True, stop=True)
            P = st.tile([128, 512], BF16, tag="Pf2")
            nc.vector.tensor_tensor(P, gb, AVU[:, jc], op=MULT)
            yf = ps_b.tile([128, 512], F32, tag="pb")
            nc.tensor.matmul(yf, wB["wf2"][:, lw], P, start=True, stop=True)
            nc.vector.scalar_tensor_tensor(
                out=x2[:, jc], in0=yf, scalar=bcol["f2b"][:, l:l + 1],
                in1=x_sb[:, jc], op0=ADD, op1=ADD)

            # ---- ln2(j) + FFN(j) ----
            ln_chunk(x2, j, pd2, r2row)
            bp2 = bcast_row(r2row, j)
            nc.vector.tensor_tensor(xn2[:, jc], bp2, x2[:, jc], op=MULT)
            cp = ps_b.tile([128, 512], F32, tag="pb")
            nc.tensor.matmul(cp, wB["wc1"][:, lw], xn2[:, jc], start=True, stop=True)
            # gelu(x) ~= silu(1.702x)/1.702; the /1.702 is folded into c2w
            nc.scalar.activation(hh[:, jc], cp, AF.Silu,
                                 bias=bcol["c1b"][:, l:l + 1], scale=GELU_A)
            c2p = ps_b.tile([128, 512], F32, tag="pb")
            nc.tensor.matmul(c2p, wB["wc2"][:, lw], hh[:, jc], start=True, stop=True)
            nc.vector.scalar_tensor_tensor(
                out=x3[:, jc], in0=c2p, scalar=bcol["c2b"][:, l:l + 1],
                in1=x2[:, jc], op0=ADD, op1=ADD)
        x_sb = x3

    # ================= final norm + output =================
    frow = st.tile([1, T], F32R, tag="r1row")
    pdf = st.tile([128, 16], F32, tag="pdf")
    o_sb = big.tile([128, T], F32, tag="e", bufs=1)
    for j in range(NT):
        jc = slice(j * 512, (j + 1) * 512)
        ln_chunk(x_sb, j, pdf, frow)
        bp = bcast_row(frow, j)
        nc.vector.scalar_tensor_tensor(
            out=o_sb[:, jc], in0=bp, scalar=last_s[:, 0:1],
            in1=x_sb[:, jc], op0=MULT, op1=MULT)
        nc.sync.dma_start(io["out"][:, jc], o_sb[:, jc])


_CACHE = {}


def _get_nc(vb_nonzero: bool):
    key = vb_nonzero
    if key in _CACHE:
        return _CACHE[key]
    nc = bacc.Bacc("TRN2", target_bir_lowering=False, debug=False)
    io = {}
    def din(name, shape, dt=F32):
        io[name] = nc.dram_tensor(name, shape, dt, kind="ExternalInput").ap()
    din("idx", (128, NS), I32)
    din("itab", (NITEMS + 1, 128))
    din("posT", (128, T))
    for nm in ("wq", "wk", "wu", "wv", "wf2", "wc1", "wc2"):
        din(nm, (L, 128, 128), BF16)
    for nm in ("ub", "qb", "kb", "c1b", "f2b", "c2b"):
        din(nm, (L, 128))
    if vb_nonzero:
        din("vbB", (L, 128, 128))
    din("sel2", (2, 128))
    din("mtri", (128, 512), BF16)
    din("ones1", (1, 128))
    din("onesc", (128, 1))
    din("ones2t", (128, 2))
    din("emb_s", (128, 1))
    din("last_s", (128, 1))
    io["out"] = nc.dram_tensor("out", (128, T), F32, kind="ExternalOutput").ap()
    with tile.TileContext(nc) as t:
        _build(t, io, vb_nonzero)
    nc.compile()
    _CACHE[key] = nc
    return nc


def _bf16(a):
    """numpy f32 -> ml_dtypes.bfloat16 array (round-to-nearest-even)."""
    import ml_dtypes
    return np.ascontiguousarray(
        np.asarray(a, dtype=np.float32).astype(ml_dtypes.bfloat16))


def _prep_maps(inputs):
    f32 = lambda a: np.ascontiguousarray(np.asarray(a, dtype=np.float32))
    log_seqs = np.asarray(inputs["log_seqs"]).astype(np.int64)
    itab = f32(inputs["item_table"])
    posT = f32(np.asarray(inputs["pos_table"], dtype=np.float32)[1:T + 1].T)
    ln1 = f32(inputs["ln1_s"]); ln2 = f32(inputs["ln2_s"])
    hstu = f32(inputs["hstu_ln_s"])
    com = {
        "itab": itab, "posT": posT,
        "wq": _bf16(ln1[:, :, None] * np.asarray(inputs["Qw"], np.float32)),
        "wk": _bf16(ln1[:, :, None] * np.asarray(inputs["Kw"], np.float32)),
        "wu": _bf16(ln1[:, :, None] * np.asarray(inputs["Uw"], np.float32)),
        "wv": _bf16(ln1[:, :, None] * np.asarray(inputs["Vw"], np.float32)),
        "wf2": _bf16(hstu[:, :, None] * np.asarray(inputs["f2w"], np.float32)),
        "wc1": _bf16(ln2[:, :, None] * np.asarray(inputs["c1w"], np.float32)),
        "wc2": _bf16(np.asarray(inputs["c2w"], np.float32) / GELU_A),
        "ub": f32(inputs["Ub"]), "qb": f32(inputs["Qb"]), "kb": f32(inputs["Kb"]),
        "c1b": f32(np.asarray(inputs["c1b"], np.float32) * GELU_A),
        "f2b": f32(inputs["f2b"]), "c2b": f32(inputs["c2b"]),
        "emb_s": f32(np.asarray(inputs["emb_ln_s"], np.float32).reshape(128, 1)),
        "last_s": f32(np.asarray(inputs["last_ln_s"], np.float32).reshape(128, 1)),
    }
    sel2 = np.zeros((2, 128), np.float32)
    sel2[0, 0:64] = 1.0
    sel2[1, 64:128] = 1.0
    com["sel2"] = sel2
    com["ones1"] = np.ones((1, 128), np.float32)
    com["onesc"] = np.ones((128, 1), np.float32)
    o2 = np.zeros((128, 2), np.float32)
    o2[0:64, 0] = 1.0
    o2[64:128, 1] = 1.0
    com["ones2t"] = o2
    mtri = (np.arange(512)[None, :] >= np.arange(128)[:, None])
    com["mtri"] = _bf16(mtri.astype(np.float32))
    vb = np.asarray(inputs["Vb"], np.float32)
    vb_nonzero = bool(np.any(vb != 0.0))
    if vb_nonzero:
        com["vbB"] = f32(np.broadcast_to(vb[:, None, :], (L, 128, 128)))
    maps = []
    for b in range(B):
        m = dict(com)
        m["idx"] = np.ascontiguousarray(
            log_seqs[b].reshape(NS, 128).T.astype(np.int32))
        maps.append(m)
    return maps, vb_nonzero


def kernel(**inputs):
    from concourse.bass_utils import run_bass_kernel_spmd
    maps, vb_nonzero = _prep_maps(inputs)
    nc = _get_nc(vb_nonzero)
    res = run_bass_kernel_spmd(nc, maps, core_ids=list(range(B)))
    out = np.stack([res.results[b]["out"].T for b in range(B)], axis=0)
    return np.ascontiguousarray(out.astype(np.float32))


if __name__ == "__main__":
    # compile-only smoke test
    nc = _get_nc(False)
    import tempfile
    from concourse.bass_utils import compile_bass_kernel
    print("NEFF:", compile_bass_kernel(nc, tempfile.mkdtemp(prefix="hstu_")))
